# revision 5
# baseline (speedup 1.0000x reference)
"""Trainium2 Bass kernel for nn_CRITTransformer (ViT-style dense transformer).

kernel(**inputs) takes FULL inputs as in reference.setup_inputs() and returns
the FULL [8, 6, 128, 128] output. Data-parallel over batch across 8
NeuronCores (1 image per core), weights replicated.

Per-core layout:
  - activations transposed: h[d=256 (2 k-tiles), s=1024]
  - attention scores^T[k, q] tiles; softmax denominators via ones-column
    appended to V (PV matmul row 32); no partition reductions
  - relative-position bias: host-expanded [128, 1920] Toeplitz cache per
    (layer, head); any k-tile's bias block is a contiguous column window;
    accumulated into scores PSUM via identity matmul
  - matmuls in float32r (full PE rate at N>=256, ~12-bit mantissa)
  - LayerNorm: stats via ones-column matmuls; rstd = exp(-0.5*ln(var+eps))
    (stays in the natural_log_exp ACT table set); stats replicated across
    partitions with K=1 matmuls
"""

import numpy as np

import concourse.bass as bass
import concourse.mybir as mybir
import concourse.tile as tile
from concourse import bacc
from concourse.bass_utils import run_bass_kernel_spmd

F32R = mybir.dt.float32r
F32 = mybir.dt.float32
AF = mybir.ActivationFunctionType
OP = mybir.AluOpType

B, C_IN, IMG, PP, D, NH, L, DFF, NCLS, MAXS = 8, 42, 128, 4, 256, 8, 4, 1024, 6, 1024
S = (IMG // PP) ** 2   # 1024
HD = D // NH           # 32
KIN = C_IN * PP * PP   # 672
KIN_PAD = 768
NKT = D // 128         # 2
NST = S // 128         # 8
VSTRIDE = NH * (HD + 1)  # 264 per s-tile in vall
EPS = 1e-6


def _build(nc, use_ln_affine, use_biases):
    def din(name, shape, dtype=F32R):
        return nc.dram_tensor(name, shape, dtype, kind="ExternalInput")

    x_unf = din("x_unf", [KIN_PAD, S])
    conv_w = din("conv_w", [KIN_PAD, D])
    pos_t = din("pos_t", [D, S])
    wq = din("wq", [L, D, D])
    wk = din("wk", [L, D, D])
    wv = din("wv", [L, D, D])
    wo = din("wo", [L, D, D])
    w1 = din("w1", [L, D, DFF])
    w2 = din("w2", [L, DFF, D])
    bcache = din("bcache", [L, NH, 128, 1920])
    ident = din("ident", [128, 128])
    ones1 = din("ones1", [1, 128])
    oavgc = din("oavgc", [128, 1])
    sel = din("sel", [8, 2 * 128])
    vinit = din("vinit", [128, NST * VSTRIDE])
    cls_w = din("cls_w", [D, NCLS * PP * PP])
    if use_biases:
        bq = din("bq", [L, D, 1], F32)
        bk = din("bk", [L, D, 1], F32)
        bv = din("bv", [L, 128, D], F32)
        bo = din("bo", [L, D, 1], F32)
        b1 = din("b1", [L, DFF, 1], F32)
        b2 = din("b2", [L, D, 1], F32)
        convb = din("convb", [D, 1], F32)
        clsb = din("clsb", [NCLS * PP * PP, 1], F32)
    if use_ln_affine:
        ln1g = din("ln1g", [L, D, 1], F32)
        ln1b = din("ln1b", [L, D, 1], F32)
        ln2g = din("ln2g", [L, D, 1], F32)
        ln2b = din("ln2b", [L, D, 1], F32)
        lnfg = din("lnfg", [D, 1], F32)
        lnfb = din("lnfb", [D, 1], F32)

    out_pl = nc.dram_tensor("out_pl", [NCLS * PP * PP, S], F32,
                            kind="ExternalOutput")

    with tile.TileContext(nc) as tc:
        with (
            tc.tile_pool(name="res", bufs=1) as res,
            tc.tile_pool(name="io", bufs=3) as io,
            tc.tile_pool(name="wp", bufs=6) as wp,
            tc.tile_pool(name="w1p", bufs=3) as w1p,
            tc.tile_pool(name="w2p", bufs=9) as w2p,
            tc.tile_pool(name="bcp", bufs=3) as bcp,
            tc.tile_pool(name="ep", bufs=4) as ep,
            tc.tile_pool(name="sgp", bufs=2) as sgp,
            tc.tile_pool(name="rowp", bufs=4) as rowp,
            tc.tile_pool(name="msc", bufs=3) as msc,
            tc.tile_pool(name="pcl", bufs=4) as pcl,
            tc.tile_pool(name="ps", bufs=2, space="PSUM") as ps,      # 2x4KB
            tc.tile_pool(name="scp", bufs=4, space="PSUM") as scp,    # 4x2KB
        ):
            # ---- constants ----
            ident_t = res.tile([128, 128], F32R, tag="ident")
            nc.sync.dma_start(ident_t[:], ident[:])
            ones1_t = res.tile([1, 128], F32R, tag="ones1")
            nc.sync.dma_start(ones1_t[:], ones1[:])
            oavgc_t = res.tile([128, 1], F32R, tag="oavgc")
            nc.sync.dma_start(oavgc_t[:], oavgc[:])
            sel_t = res.tile([8, 2 * 128], F32R, tag="sel")
            nc.sync.dma_start(sel_t[:], sel[:])
            epst = res.tile([128, 1], F32, tag="eps")
            nc.vector.memset(epst[:], EPS)

            h = [res.tile([128, S], F32R, tag=f"h{kt}", name=f"h{kt}") for kt in range(NKT)]
            xr = [res.tile([128, S], F32R, tag=f"xr{kt}", name=f"xr{kt}") for kt in range(NKT)]
            qt = [res.tile([128, S], F32R, tag=f"qt{c}", name=f"qt{c}") for c in range(NKT)]
            ktsb = [res.tile([128, S], F32R, tag=f"kt{c}", name=f"ktsb{c}") for c in range(NKT)]
            oall = [res.tile([128, S], F32R, tag=f"oall{c}", name=f"oall{c}") for c in range(NKT)]
            vall = res.tile([128, NST * VSTRIDE], F32R, tag="vall")
            zall = res.tile([8, S], F32R, tag="zall")
            zrall = res.tile([8, S], F32R, tag="zrall")

            nc.sync.dma_start(vall[:], vinit[:])

            def mm_halves(psum, lhsT, rhs, start, stop, tile_position=None):
                for hf in range(2):
                    nc.tensor.matmul(
                        psum[:, hf * 512:(hf + 1) * 512], lhsT,
                        rhs[:, hf * 512:(hf + 1) * 512],
                        start=start, stop=stop, skip_group_check=True,
                        tile_position=tile_position)

            def pcol(src_ap):
                t = pcl.tile([128, 1], F32, tag="pcol", name="pcol")
                nc.sync.dma_start(t[:src_ap.shape[0], :], src_ap)
                return t

            # ================= patch embedding =================
            for c in range(NKT):
                cps = ps.tile([128, S], F32, tag="pv")
                for kt in range(6):
                    xt_ = io.tile([128, S], F32R, tag="io")
                    nc.sync.dma_start(xt_[:], x_unf[kt * 128:(kt + 1) * 128, :])
                    wt = wp.tile([128, 128], F32R, tag="wc")
                    nc.sync.dma_start(
                        wt[:], conv_w[kt * 128:(kt + 1) * 128,
                                      c * 128:(c + 1) * 128])
                    mm_halves(cps, wt[:], xt_[:], start=(kt == 0), stop=False)
                post = io.tile([128, S], F32R, tag="io")
                nc.sync.dma_start(post[:], pos_t[c * 128:(c + 1) * 128, :])
                mm_halves(cps, ident_t[:], post[:], start=False, stop=True)
                if use_biases:
                    nc.scalar.activation(h[c][:], cps[:], AF.Identity,
                                         bias=pcol(convb[c * 128:(c + 1) * 128, :])[:])
                else:
                    nc.scalar.copy(h[c][:], cps[:])

            # ================= layernorm helper =================
            def layernorm(xt, out_t, g_ap, b_ap):
                mrow_ps = ps.tile([1, S], F32, tag="pv")
                qrow_ps = ps.tile([1, S], F32, tag="pv")
                for kt in range(NKT):
                    sq = msc.tile([128, S], F32R, tag="sq")
                    nc.vector.tensor_tensor(sq[:], xt[kt][:], xt[kt][:],
                                            OP.mult)
                    mm_halves(mrow_ps, oavgc_t[:], xt[kt][:],
                              start=(kt == 0), stop=(kt == NKT - 1))
                    mm_halves(qrow_ps, oavgc_t[:], sq[:],
                              start=(kt == 0), stop=(kt == NKT - 1))
                mrow = rowp.tile([1, S], F32R, tag="row")
                nc.vector.tensor_copy(mrow[:], mrow_ps[:])
                m2 = rowp.tile([1, S], F32, tag="row")
                nc.vector.tensor_tensor(m2[:], mrow[:], mrow[:], OP.mult)
                var = rowp.tile([1, S], F32, tag="row")
                nc.vector.tensor_tensor(var[:], qrow_ps[:], m2[:], OP.subtract)
                rrow = rowp.tile([1, S], F32R, tag="row")
                nc.scalar.activation(rrow[:], var[:], AF.Ln, bias=epst[0:1, :])
                nc.scalar.activation(rrow[:], rrow[:], AF.Exp, scale=-0.5)
                arow = rowp.tile([1, S], F32R, tag="row")
                nc.vector.scalar_tensor_tensor(arow[:], mrow[:], -1.0,
                                               rrow[:], OP.mult, OP.mult)
                rrep = ps.tile([128, S], F32, tag="pv")
                mm_halves(rrep, ones1_t[:], rrow[:], start=True, stop=True)
                arep = ps.tile([128, S], F32, tag="pv")
                mm_halves(arep, ones1_t[:], arow[:], start=True, stop=True)
                for kt in range(NKT):
                    u = msc.tile([128, S], F32, tag="sq")
                    nc.vector.tensor_tensor(u[:], xt[kt][:], rrep[:], OP.mult)
                    if g_ap is None:
                        nc.vector.tensor_tensor(out_t[kt][:], u[:], arep[:],
                                                OP.add)
                    else:
                        u2 = msc.tile([128, S], F32, tag="sq")
                        nc.vector.tensor_tensor(u2[:], u[:], arep[:], OP.add)
                        nc.scalar.activation(out_t[kt][:], u2[:], AF.Identity,
                                             scale=pcol(g_ap[kt])[:],
                                             bias=pcol(b_ap[kt])[:])

            # ================= transformer layers =================
            for l in range(L):
                # ---- Q^T, K^T ----
                for c in range(NKT):
                    qps = ps.tile([128, S], F32, tag="pv")
                    kps = ps.tile([128, S], F32, tag="pv")
                    for kt in range(NKT):
                        wqt = wp.tile([128, 128], F32R, tag="wc")
                        nc.sync.dma_start(
                            wqt[:], wq[l, kt * 128:(kt + 1) * 128,
                                       c * 128:(c + 1) * 128])
                        mm_halves(qps, wqt[:], h[kt][:], start=(kt == 0),
                                  stop=(kt == NKT - 1))
                        wkt = wp.tile([128, 128], F32R, tag="wc")
                        nc.sync.dma_start(
                            wkt[:], wk[l, kt * 128:(kt + 1) * 128,
                                       c * 128:(c + 1) * 128])
                        mm_halves(kps, wkt[:], h[kt][:], start=(kt == 0),
                                  stop=(kt == NKT - 1))
                    if use_biases:
                        nc.scalar.activation(
                            qt[c][:], qps[:], AF.Identity,
                            bias=pcol(bq[l, c * 128:(c + 1) * 128, :])[:])
                        nc.scalar.activation(
                            ktsb[c][:], kps[:], AF.Identity,
                            bias=pcol(bk[l, c * 128:(c + 1) * 128, :])[:])
                    else:
                        nc.scalar.copy(qt[c][:], qps[:])
                        nc.vector.tensor_copy(ktsb[c][:], kps[:])
                # ---- V (s-partition layout, interleaved ones cols) ----
                wvt = [w1p.tile([128, D], F32R, tag="wv", name=f"wv{i}") for i in range(NKT)]
                for kt in range(NKT):
                    nc.sync.dma_start(wvt[kt][:],
                                      wv[l, kt * 128:(kt + 1) * 128, :])
                if use_biases:
                    bvt = msc.tile([128, D], F32, tag="bvrep")
                    nc.sync.dma_start(bvt[:], bv[l])
                for st in range(NST):
                    vps = scp.tile([128, D], F32, tag="sc")
                    for kt in range(NKT):
                        nc.tensor.matmul(
                            vps[:], h[kt][:, st * 128:(st + 1) * 128],
                            wvt[kt][:], start=(kt == 0),
                            stop=(kt == NKT - 1), skip_group_check=True)
                    base = st * VSTRIDE
                    dst = bass.AP(vall.tensor, vall[:].offset + base,
                                  [list(vall[:].ap[0]), [HD + 1, NH], [1, HD]])
                    if use_biases:
                        nc.vector.tensor_tensor(
                            dst, vps[:].rearrange("p (a b) -> p a b", a=NH),
                            bvt[:].rearrange("p (a b) -> p a b", a=NH), OP.add)
                    else:
                        nc.vector.tensor_copy(
                            dst, vps[:].rearrange("p (a b) -> p a b", a=NH))

                # ---- attention ----
                for g in range(4):          # head pairs
                    h0, h1 = 2 * g, 2 * g + 1
                    chunk = h0 // 4
                    r0 = 32 * (h0 % 4)
                    bct = []
                    for j in range(2):
                        t = bcp.tile([128, 1920], F32R, tag="bc", name="bct")
                        nc.sync.dma_start(t[:], bcache[l, 2 * g + j])
                        bct.append(t)
                    pvps = [ps.tile([33, S], F32, tag="pv", name=f"pv{i}")
                            for i in range(2)]
                    for qh in range(2):
                        for kt8 in range(NST):
                            scts = []
                            for j in range(2):
                                jr = r0 + 32 * j
                                sct = scp.tile([128, 512], F32, tag="sc",
                                               name="sct")
                                nc.tensor.matmul(
                                    sct[:],
                                    ktsb[chunk][jr:jr + 32,
                                                kt8 * 128:(kt8 + 1) * 128],
                                    qt[chunk][jr:jr + 32,
                                              qh * 512:(qh + 1) * 512],
                                    start=True, stop=False,
                                    skip_group_check=True,
                                    tile_position=(jr, 0))
                                scts.append(sct)
                            for j in range(2):
                                nc.tensor.matmul(
                                    scts[j][:], ident_t[:],
                                    bct[j][:, (7 - kt8) * 128 + qh * 512:
                                           (7 - kt8) * 128 + qh * 512 + 512],
                                    start=False, stop=True,
                                    skip_group_check=True)
                            for j in range(2):
                                hh = 2 * g + j
                                et = ep.tile([128, 512], F32R, tag="e",
                                             name="et")
                                nc.scalar.activation(et[:], scts[j][:], AF.Exp)
                                vsl = vall[:, kt8 * VSTRIDE + hh * (HD + 1):
                                           kt8 * VSTRIDE + hh * (HD + 1) +
                                           HD + 1]
                                nc.tensor.matmul(
                                    pvps[j][:, qh * 512:(qh + 1) * 512],
                                    vsl, et[:],
                                    start=(kt8 == 0), stop=(kt8 == NST - 1),
                                    skip_group_check=True)
                    for j in range(2):
                        hh = 2 * g + j
                        stg = sgp.tile([33, S], F32R, tag="stage", name="stg")
                        nc.vector.tensor_copy(stg[:], pvps[j][:])
                        nc.sync.dma_start(
                            oall[hh // 4][32 * (hh % 4):32 * (hh % 4) + 32, :],
                            stg[0:32, :])
                        nc.sync.dma_start(zall[hh:hh + 1, :], stg[32:33, :])
                zrtmp = rowp.tile([8, S], F32, tag="zrtmp")
                nc.vector.reciprocal_approx_fast(zrtmp[:], zall[:].bitcast(F32))
                nc.vector.tensor_copy(zrall[:], zrtmp[:])
                for c in range(NKT):
                    zrep = ps.tile([128, S], F32, tag="pv")
                    mm_halves(zrep, sel_t[:, c * 128:(c + 1) * 128],
                              zrall[:], start=True, stop=True)
                    nc.vector.tensor_tensor(oall[c][:], oall[c][:], zrep[:],
                                            OP.mult)
                # ---- wo + residual ----
                for c in range(NKT):
                    aps = ps.tile([128, S], F32, tag="pv")
                    for kt in range(NKT):
                        wot = wp.tile([128, 128], F32R, tag="wc")
                        nc.sync.dma_start(
                            wot[:], wo[l, kt * 128:(kt + 1) * 128,
                                       c * 128:(c + 1) * 128])
                        mm_halves(aps, wot[:], oall[kt][:], start=(kt == 0),
                                  stop=(kt == NKT - 1))
                    if use_biases:
                        nc.vector.scalar_tensor_tensor(
                            xr[c][:], aps[:],
                            pcol(bo[l, c * 128:(c + 1) * 128, :])[:],
                            h[c][:], OP.add, OP.add)
                    else:
                        nc.vector.tensor_tensor(xr[c][:], aps[:], h[c][:],
                                                OP.add)
                if use_ln_affine:
                    layernorm(xr, h,
                              [ln1g[l, k * 128:(k + 1) * 128, :] for k in range(NKT)],
                              [ln1b[l, k * 128:(k + 1) * 128, :] for k in range(NKT)])
                else:
                    layernorm(xr, h, None, None)

                # ---- FFN ----
                w1t = [w1p.tile([128, DFF], F32R, tag="w1", name=f"w1t{i}") for i in range(NKT)]
                for kt in range(NKT):
                    nc.sync.dma_start(w1t[kt][:],
                                      w1[l, kt * 128:(kt + 1) * 128, :])
                w2t = [w2p.tile([128, D], F32R, tag="w2", name=f"w2t{i}") for i in range(DFF // 128)]
                for kt in range(DFF // 128):
                    nc.sync.dma_start(w2t[kt][:],
                                      w2[l, kt * 128:(kt + 1) * 128, :])
                fps = [ps.tile([128, S], F32, tag="pv", name=f"fps{i}") for i in range(NKT)]
                for ch in range(DFF // 128):
                    gh = [scp.tile([128, 512], F32, tag="sc", name=f"gh{i}") for i in range(2)]
                    for kt in range(NKT):
                        for hf in range(2):
                            nc.tensor.matmul(
                                gh[hf][:],
                                w1t[kt][:, ch * 128:(ch + 1) * 128],
                                h[kt][:, hf * 512:(hf + 1) * 512],
                                start=(kt == 0), stop=(kt == NKT - 1),
                                skip_group_check=True)
                    gt = io.tile([128, S], F32R, tag="io")
                    b1c = (pcol(b1[l, ch * 128:(ch + 1) * 128, :])
                           if use_biases else None)
                    for hf in range(2):
                        dstg = gt[:, hf * 512:(hf + 1) * 512]
                        if ch % 2 == 0:
                            nc.scalar.activation(
                                dstg, gh[hf][:], AF.Relu,
                                bias=(b1c[:] if b1c is not None else 0.0))
                        else:
                            if b1c is not None:
                                nc.vector.tensor_scalar(
                                    dstg, gh[hf][:], b1c[:], 0.0,
                                    OP.add, OP.max)
                            else:
                                nc.vector.tensor_scalar_max(dstg, gh[hf][:],
                                                            0.0)
                    for c in range(NKT):
                        mm_halves(fps[c], w2t[ch][:, c * 128:(c + 1) * 128],
                                  gt[:], start=(ch == 0),
                                  stop=(ch == DFF // 128 - 1))
                for c in range(NKT):
                    if use_biases:
                        nc.vector.scalar_tensor_tensor(
                            xr[c][:], fps[c][:],
                            pcol(b2[l, c * 128:(c + 1) * 128, :])[:],
                            h[c][:], OP.add, OP.add)
                    else:
                        nc.vector.tensor_tensor(xr[c][:], fps[c][:], h[c][:],
                                                OP.add)
                if use_ln_affine:
                    layernorm(xr, h,
                              [ln2g[l, k * 128:(k + 1) * 128, :] for k in range(NKT)],
                              [ln2b[l, k * 128:(k + 1) * 128, :] for k in range(NKT)])
                else:
                    layernorm(xr, h, None, None)

            # ================= final LN + classifier =================
            hf_t = [msc.tile([128, S], F32R, tag="sq", name=f"hf{i}") for i in range(NKT)]
            if use_ln_affine:
                layernorm(h, hf_t,
                          [lnfg[k * 128:(k + 1) * 128, :] for k in range(NKT)],
                          [lnfb[k * 128:(k + 1) * 128, :] for k in range(NKT)])
            else:
                layernorm(h, hf_t, None, None)
            cps = ps.tile([NCLS * PP * PP, S], F32, tag="pv")
            for kt in range(NKT):
                cwt = wp.tile([128, NCLS * PP * PP], F32R, tag="wc")
                nc.sync.dma_start(cwt[:], cls_w[kt * 128:(kt + 1) * 128, :])
                mm_halves(cps, cwt[:], hf_t[kt][:], start=(kt == 0),
                          stop=(kt == NKT - 1))
            outt = io.tile([NCLS * PP * PP, S], F32, tag="io")
            if use_biases:
                nc.scalar.activation(outt[:], cps[:], AF.Identity,
                                     bias=pcol(clsb[:])[:])
            else:
                nc.scalar.copy(outt[:], cps[:])
            nc.sync.dma_start(out_pl[:], outt[:])


def _prep_host(inputs):
    f = lambda a: np.ascontiguousarray(np.asarray(a), dtype=np.float32)
    x = f(inputs["x"])
    conv_w = f(inputs["conv_w"])
    pos = f(inputs["pos_embed"])
    rpb = f(inputs["rpb"])

    xs = []
    for b in range(B):
        xb = x[b].reshape(C_IN, IMG // PP, PP, IMG // PP, PP)
        xb = xb.transpose(0, 2, 4, 1, 3).reshape(KIN, S)
        xp = np.zeros((KIN_PAD, S), np.float32)
        xp[:KIN] = xb
        xs.append(xp)

    w = {}
    cw = conv_w.reshape(D, C_IN, PP, PP).transpose(1, 2, 3, 0).reshape(KIN, D)
    cwp = np.zeros((KIN_PAD, D), np.float32)
    cwp[:KIN] = cw
    w["conv_w"] = cwp
    w["pos_t"] = f(pos.reshape(S, D).T)
    scale = 1.0 / np.sqrt(np.float32(HD))
    w["wq"] = f(np.transpose(inputs["wq"], (0, 2, 1)) * scale)
    w["wk"] = f(np.transpose(inputs["wk"], (0, 2, 1)))
    w["wv"] = f(np.transpose(inputs["wv"], (0, 2, 1)))
    w["wo"] = f(np.transpose(inputs["wo"], (0, 2, 1)))
    w["w1"] = f(np.transpose(inputs["w1"], (0, 2, 1)))
    w["w2"] = f(np.transpose(inputs["w2"], (0, 2, 1)))
    bc = np.zeros((L, NH, 128, 1920), np.float32)
    for l in range(L):
        for hh in range(NH):
            th = np.ascontiguousarray(rpb[l, :, hh])
            bc[l, hh] = np.lib.stride_tricks.as_strided(
                th[127:], shape=(128, 1920), strides=(-4, 4))
    w["bcache"] = bc
    w["ident"] = np.eye(128, dtype=np.float32)
    w["ones1"] = np.ones((1, 128), np.float32)
    w["oavgc"] = np.full((128, 1), 1.0 / D, np.float32)
    selw = np.zeros((8, 2 * 128), np.float32)
    for c in range(2):
        for p in range(128):
            selw[4 * c + p // 32, c * 128 + p] = 1.0
    w["sel"] = selw
    w["cls_w"] = f(inputs["cls_w"].T)
    vinit = np.zeros((128, NST * VSTRIDE), np.float32)
    for st in range(NST):
        for hh in range(NH):
            vinit[:, st * VSTRIDE + hh * (HD + 1) + HD] = 1.0
    w["vinit"] = vinit

    use_biases = any(
        np.abs(f(inputs[k])).max() > 0
        for k in ("bq", "bk", "bv", "bo", "b1", "b2", "conv_b", "cls_b"))
    use_ln_affine = not (
        np.allclose(f(inputs["ln1_s"]), 1.0)
        and np.allclose(f(inputs["ln2_s"]), 1.0)
        and np.allclose(f(inputs["lnf_s"]), 1.0)
        and np.abs(f(inputs["ln1_b"])).max() == 0
        and np.abs(f(inputs["ln2_b"])).max() == 0
        and np.abs(f(inputs["lnf_b"])).max() == 0)
    if use_biases:
        w["bq"] = f(inputs["bq"]).reshape(L, D, 1)
        w["bk"] = f(inputs["bk"]).reshape(L, D, 1)
        w["bv"] = np.ascontiguousarray(
            np.broadcast_to(f(inputs["bv"])[:, None, :], (L, 128, D)))
        w["bo"] = f(inputs["bo"]).reshape(L, D, 1)
        w["b1"] = f(inputs["b1"]).reshape(L, DFF, 1)
        w["b2"] = f(inputs["b2"]).reshape(L, D, 1)
        w["convb"] = f(inputs["conv_b"]).reshape(D, 1)
        w["clsb"] = f(inputs["cls_b"]).reshape(NCLS * PP * PP, 1)
    if use_ln_affine:
        w["ln1g"] = f(inputs["ln1_s"]).reshape(L, D, 1)
        w["ln1b"] = f(inputs["ln1_b"]).reshape(L, D, 1)
        w["ln2g"] = f(inputs["ln2_s"]).reshape(L, D, 1)
        w["ln2b"] = f(inputs["ln2_b"]).reshape(L, D, 1)
        w["lnfg"] = f(inputs["lnf_s"]).reshape(D, 1)
        w["lnfb"] = f(inputs["lnf_b"]).reshape(D, 1)
    return w, xs, use_ln_affine, use_biases


_RUN_KWARGS = {}


def kernel(**inputs):
    w, xs, use_ln_affine, use_biases = _prep_host(inputs)
    nc = bacc.Bacc("TRN2")
    _build(nc, use_ln_affine, use_biases)
    nc.finalize()
    in_maps = [dict(w, x_unf=xs[b]) for b in range(B)]
    res = run_bass_kernel_spmd(nc, in_maps, core_ids=list(range(B)),
                               **_RUN_KWARGS)
    kernel.last_result = res
    out = np.empty((B, NCLS, IMG, IMG), np.float32)
    for b in range(B):
        pl = res.results[b]["out_pl"]
        pl = pl.reshape(NCLS, PP, PP, IMG // PP, IMG // PP)
        out[b] = pl.transpose(0, 3, 1, 4, 2).reshape(NCLS, IMG, IMG)
    return out


# revision 9
# speedup vs baseline: 1.1051x; 1.1051x over previous
"""Trainium2 Bass kernel for nn_CRITTransformer (ViT-style dense transformer).

kernel(**inputs) takes FULL inputs as in reference.setup_inputs() and returns
the FULL [8, 6, 128, 128] output. Data-parallel over batch across 8
NeuronCores (1 image per core), weights replicated.

Per-core layout:
  - activations transposed: h[d=256 (2 k-tiles), s=1024]
  - attention scores^T[k, q] tiles; softmax denominators via ones-column
    appended to V (PV matmul row 32); no partition reductions
  - relative-position bias: host-expanded [128, 1920] Toeplitz cache per
    (layer, head); any k-tile's bias block is a contiguous column window;
    accumulated into scores PSUM via identity matmul
  - matmuls in float32r (full PE rate at N>=256, ~12-bit mantissa)
  - LayerNorm: stats via ones-column matmuls; rstd = exp(-0.5*ln(var+eps))
    (stays in the natural_log_exp ACT table set); stats replicated across
    partitions with K=1 matmuls
"""

import numpy as np

import concourse.bass as bass
import concourse.mybir as mybir
import concourse.tile as tile
from concourse import bacc
from concourse.bass_utils import run_bass_kernel_spmd

F32R = mybir.dt.float32r
F32 = mybir.dt.float32
BF16 = mybir.dt.bfloat16
AF = mybir.ActivationFunctionType
OP = mybir.AluOpType

B, C_IN, IMG, PP, D, NH, L, DFF, NCLS, MAXS = 8, 42, 128, 4, 256, 8, 4, 1024, 6, 1024
S = (IMG // PP) ** 2   # 1024
HD = D // NH           # 32
KIN = C_IN * PP * PP   # 672
KIN_PAD = 768
NKT = D // 128         # 2
NST = S // 128         # 8
VSTRIDE = NH * (HD + 1)  # 264 per s-tile in vall
EPS = 1e-6


def _build(nc, use_ln_affine, use_biases):
    def din(name, shape, dtype=F32R):
        return nc.dram_tensor(name, shape, dtype, kind="ExternalInput")

    x_unf = din("x_unf", [KIN_PAD, S])
    conv_w = din("conv_w", [KIN_PAD, D])
    pos_t = din("pos_t", [D, S])
    wq = din("wq", [L, D, D], BF16)
    wk = din("wk", [L, D, D], BF16)
    wv = din("wv", [L, D, D], BF16)
    wo = din("wo", [L, D, D], BF16)
    w1 = din("w1", [L, D, DFF], BF16)
    w2 = din("w2", [L, DFF, D], BF16)
    bcache = din("bcache", [L, NH, 128, 1920], BF16)
    ident = din("ident", [128, 128], BF16)
    ident32 = din("ident32", [128, 128])
    ones1 = din("ones1", [1, 128])
    oavgc = din("oavgc", [128, 1])
    sel = din("sel", [8, 2 * 128])
    vinit = din("vinit", [128, NST * VSTRIDE], BF16)
    cls_w = din("cls_w", [D, NCLS * PP * PP])
    if use_biases:
        bq = din("bq", [L, D, 1], F32)
        bk = din("bk", [L, D, 1], F32)
        bv = din("bv", [L, 128, D], F32)
        bo = din("bo", [L, D, 1], F32)
        b1 = din("b1", [L, DFF, 1], F32)
        b2 = din("b2", [L, D, 1], F32)
        convb = din("convb", [D, 1], F32)
        clsb = din("clsb", [NCLS * PP * PP, 1], F32)
    if use_ln_affine:
        ln1g = din("ln1g", [L, D, 1], F32)
        ln1b = din("ln1b", [L, D, 1], F32)
        ln2g = din("ln2g", [L, D, 1], F32)
        ln2b = din("ln2b", [L, D, 1], F32)
        lnfg = din("lnfg", [D, 1], F32)
        lnfb = din("lnfb", [D, 1], F32)

    out_pl = nc.dram_tensor("out_pl", [NCLS * PP * PP, S], F32,
                            kind="ExternalOutput")

    with tile.TileContext(nc) as tc:
        with (
            tc.tile_pool(name="res", bufs=1) as res,
            tc.tile_pool(name="io", bufs=3) as io,
            tc.tile_pool(name="wp", bufs=6) as wp,
            tc.tile_pool(name="w1p", bufs=3) as w1p,
            tc.tile_pool(name="w2p", bufs=9) as w2p,
            tc.tile_pool(name="bcp", bufs=3) as bcp,
            tc.tile_pool(name="ep", bufs=4) as ep,
            tc.tile_pool(name="sgp", bufs=2) as sgp,
            tc.tile_pool(name="rowp", bufs=4) as rowp,
            tc.tile_pool(name="msc", bufs=3) as msc,
            tc.tile_pool(name="pcl", bufs=4) as pcl,
            tc.tile_pool(name="ps", bufs=2, space="PSUM") as ps,      # 2x4KB
            tc.tile_pool(name="scp", bufs=4, space="PSUM") as scp,    # 4x2KB
        ):
            # ---- constants ----
            ident_t = res.tile([128, 128], BF16, tag="ident")
            nc.sync.dma_start(ident_t[:], ident[:])
            ident32_t = res.tile([128, 128], F32R, tag="ident32")
            nc.sync.dma_start(ident32_t[:], ident32[:])
            ones1_t = res.tile([1, 128], F32R, tag="ones1")
            nc.sync.dma_start(ones1_t[:], ones1[:])
            oavgc_t = res.tile([128, 1], F32R, tag="oavgc")
            nc.sync.dma_start(oavgc_t[:], oavgc[:])
            sel_t = res.tile([8, 2 * 128], F32R, tag="sel")
            nc.sync.dma_start(sel_t[:], sel[:])
            epst = res.tile([128, 1], F32, tag="eps")
            nc.vector.memset(epst[:], EPS)

            h = [res.tile([128, S], F32R, tag=f"h{kt}", name=f"h{kt}") for kt in range(NKT)]
            h16 = [res.tile([128, S], BF16, tag=f"h16{kt}", name=f"h16_{kt}") for kt in range(NKT)]
            xr = [res.tile([128, S], F32R, tag=f"xr{kt}", name=f"xr{kt}") for kt in range(NKT)]
            qt = [res.tile([128, S], BF16, tag=f"qt{c}", name=f"qt{c}") for c in range(NKT)]
            ktsb = [res.tile([128, S], BF16, tag=f"kt{c}", name=f"ktsb{c}") for c in range(NKT)]
            oall = [res.tile([128, S], BF16, tag=f"oall{c}", name=f"oall{c}") for c in range(NKT)]
            vall = res.tile([128, NST * VSTRIDE], BF16, tag="vall")
            zall = res.tile([8, S], BF16, tag="zall")
            zrall = res.tile([8, S], F32R, tag="zrall")

            nc.sync.dma_start(vall[:], vinit[:])

            def mm_halves(psum, lhsT, rhs, start, stop, tile_position=None):
                for hf in range(2):
                    nc.tensor.matmul(
                        psum[:, hf * 512:(hf + 1) * 512], lhsT,
                        rhs[:, hf * 512:(hf + 1) * 512],
                        start=start, stop=stop, skip_group_check=True,
                        tile_position=tile_position)


            def pcol(src_ap):
                t = pcl.tile([128, 1], F32, tag="pcol", name="pcol")
                nc.sync.dma_start(t[:src_ap.shape[0], :], src_ap)
                return t

            # ================= patch embedding =================
            for c in range(NKT):
                cps = ps.tile([128, S], F32, tag="pv")
                for kt in range(6):
                    xt_ = io.tile([128, S], F32R, tag="io")
                    nc.sync.dma_start(xt_[:], x_unf[kt * 128:(kt + 1) * 128, :])
                    wt = wp.tile([128, 128], F32R, tag="wc")
                    nc.sync.dma_start(
                        wt[:], conv_w[kt * 128:(kt + 1) * 128,
                                      c * 128:(c + 1) * 128])
                    mm_halves(cps, wt[:], xt_[:], start=(kt == 0), stop=False)
                post = io.tile([128, S], F32R, tag="io")
                nc.sync.dma_start(post[:], pos_t[c * 128:(c + 1) * 128, :])
                mm_halves(cps, ident32_t[:], post[:], start=False, stop=True)
                if use_biases:
                    nc.scalar.activation(h[c][:], cps[:], AF.Identity,
                                         bias=pcol(convb[c * 128:(c + 1) * 128, :])[:])
                else:
                    nc.scalar.copy(h[c][:], cps[:])
                nc.vector.tensor_copy(h16[c][:], h[c][:])

            # ================= layernorm helper =================
            def layernorm(xt, out_t, g_ap, b_ap):
                mrow_ps = ps.tile([1, S], F32, tag="pv")
                qrow_ps = ps.tile([1, S], F32, tag="pv")
                for kt in range(NKT):
                    sq = msc.tile([128, S], F32R, tag="sq")
                    nc.vector.tensor_tensor(sq[:], xt[kt][:], xt[kt][:],
                                            OP.mult)
                    mm_halves(mrow_ps, oavgc_t[:], xt[kt][:],
                              start=(kt == 0), stop=(kt == NKT - 1))
                    mm_halves(qrow_ps, oavgc_t[:], sq[:],
                              start=(kt == 0), stop=(kt == NKT - 1))
                mrow = rowp.tile([1, S], F32R, tag="row")
                nc.vector.tensor_copy(mrow[:], mrow_ps[:])
                m2 = rowp.tile([1, S], F32, tag="row")
                nc.vector.tensor_tensor(m2[:], mrow[:], mrow[:], OP.mult)
                var = rowp.tile([1, S], F32, tag="row")
                nc.vector.tensor_tensor(var[:], qrow_ps[:], m2[:], OP.subtract)
                rrow = rowp.tile([1, S], F32R, tag="row")
                nc.scalar.activation(rrow[:], var[:], AF.Ln, bias=epst[0:1, :])
                nc.scalar.activation(rrow[:], rrow[:], AF.Exp, scale=-0.5)
                arow = rowp.tile([1, S], F32R, tag="row")
                nc.vector.scalar_tensor_tensor(arow[:], mrow[:], -1.0,
                                               rrow[:], OP.mult, OP.mult)
                rrep = ps.tile([128, S], F32, tag="pv")
                mm_halves(rrep, ones1_t[:], rrow[:], start=True, stop=True)
                arep = ps.tile([128, S], F32, tag="pv")
                mm_halves(arep, ones1_t[:], arow[:], start=True, stop=True)
                for kt in range(NKT):
                    u = msc.tile([128, S], F32, tag="sq")
                    nc.vector.tensor_tensor(u[:], xt[kt][:], rrep[:], OP.mult)
                    if g_ap is None:
                        nc.vector.tensor_tensor(out_t[kt][:], u[:], arep[:],
                                                OP.add)
                    else:
                        u2 = msc.tile([128, S], F32, tag="sq")
                        nc.vector.tensor_tensor(u2[:], u[:], arep[:], OP.add)
                        nc.scalar.activation(out_t[kt][:], u2[:], AF.Identity,
                                             scale=pcol(g_ap[kt])[:],
                                             bias=pcol(b_ap[kt])[:])

            # ================= transformer layers =================
            for l in range(L):
                # ---- Q^T, K^T ----
                for c in range(NKT):
                    qps = ps.tile([128, S], F32, tag="pv")
                    kps = ps.tile([128, S], F32, tag="pv")
                    for kt in range(NKT):
                        wqt = wp.tile([128, 128], BF16, tag="wc")
                        nc.sync.dma_start(
                            wqt[:], wq[l, kt * 128:(kt + 1) * 128,
                                       c * 128:(c + 1) * 128])
                        mm_halves(qps, wqt[:], h16[kt][:], start=(kt == 0),
                                stop=(kt == NKT - 1))
                        wkt = wp.tile([128, 128], BF16, tag="wc")
                        nc.sync.dma_start(
                            wkt[:], wk[l, kt * 128:(kt + 1) * 128,
                                       c * 128:(c + 1) * 128])
                        mm_halves(kps, wkt[:], h16[kt][:], start=(kt == 0),
                                stop=(kt == NKT - 1))
                    if use_biases:
                        nc.scalar.activation(
                            qt[c][:], qps[:], AF.Identity,
                            bias=pcol(bq[l, c * 128:(c + 1) * 128, :])[:])
                        nc.scalar.activation(
                            ktsb[c][:], kps[:], AF.Identity,
                            bias=pcol(bk[l, c * 128:(c + 1) * 128, :])[:])
                    else:
                        nc.scalar.copy(qt[c][:], qps[:])
                        nc.vector.tensor_copy(ktsb[c][:], kps[:])
                # ---- V (s-partition layout, interleaved ones cols) ----
                wvt = [w1p.tile([128, D], BF16, tag="wv", name=f"wv{i}") for i in range(NKT)]
                for kt in range(NKT):
                    nc.sync.dma_start(wvt[kt][:],
                                      wv[l, kt * 128:(kt + 1) * 128, :])
                if use_biases:
                    bvt = msc.tile([128, D], F32, tag="bvrep")
                    nc.sync.dma_start(bvt[:], bv[l])
                for st in range(NST):
                    vps = scp.tile([128, D], F32, tag="sc")
                    for kt in range(NKT):
                        nc.tensor.matmul(
                            vps[:], h16[kt][:, st * 128:(st + 1) * 128],
                            wvt[kt][:], start=(kt == 0),
                            stop=(kt == NKT - 1), skip_group_check=True)
                    base = st * VSTRIDE
                    dst = bass.AP(vall.tensor, vall[:].offset + base,
                                  [list(vall[:].ap[0]), [HD + 1, NH], [1, HD]])
                    if use_biases:
                        nc.vector.tensor_tensor(
                            dst, vps[:].rearrange("p (a b) -> p a b", a=NH),
                            bvt[:].rearrange("p (a b) -> p a b", a=NH), OP.add)
                    else:
                        nc.vector.tensor_copy(
                            dst, vps[:].rearrange("p (a b) -> p a b", a=NH))

                # ---- attention ----
                for g in range(4):          # head pairs
                    h0, h1 = 2 * g, 2 * g + 1
                    chunk = h0 // 4
                    r0 = 32 * (h0 % 4)
                    bct = []
                    for j in range(2):
                        t = bcp.tile([128, 1920], BF16, tag="bc", name="bct")
                        nc.sync.dma_start(t[:], bcache[l, 2 * g + j])
                        bct.append(t)
                    pvps = [ps.tile([33, S], F32, tag="pv", name=f"pv{i}")
                            for i in range(2)]
                    for qh in range(2):
                        for kt8 in range(NST):
                            scts = []
                            for j in range(2):
                                jr = r0 + 32 * j
                                sct = scp.tile([128, 512], F32, tag="sc",
                                               name="sct")
                                nc.tensor.matmul(
                                    sct[:],
                                    ktsb[chunk][jr:jr + 32,
                                                kt8 * 128:(kt8 + 1) * 128],
                                    qt[chunk][jr:jr + 32,
                                              qh * 512:(qh + 1) * 512],
                                    start=True, stop=False,
                                    skip_group_check=True,
                                    tile_position=(jr, 0))
                                scts.append(sct)
                            for j in range(2):
                                nc.tensor.matmul(
                                    scts[j][:], ident_t[:],
                                    bct[j][:, (7 - kt8) * 128 + qh * 512:
                                           (7 - kt8) * 128 + qh * 512 + 512],
                                    start=False, stop=True,
                                    skip_group_check=True)
                            for j in range(2):
                                hh = 2 * g + j
                                et = ep.tile([128, 512], BF16, tag="e",
                                             name="et")
                                nc.scalar.activation(et[:], scts[j][:], AF.Exp)
                                vsl = vall[:, kt8 * VSTRIDE + hh * (HD + 1):
                                           kt8 * VSTRIDE + hh * (HD + 1) +
                                           HD + 1]
                                nc.tensor.matmul(
                                    pvps[j][:, qh * 512:(qh + 1) * 512],
                                    vsl, et[:],
                                    start=(kt8 == 0), stop=(kt8 == NST - 1),
                                    skip_group_check=True)
                    for j in range(2):
                        hh = 2 * g + j
                        stg = sgp.tile([33, S], BF16, tag="stage", name="stg")
                        nc.vector.tensor_copy(stg[:], pvps[j][:])
                        nc.sync.dma_start(
                            oall[hh // 4][32 * (hh % 4):32 * (hh % 4) + 32, :],
                            stg[0:32, :])
                        nc.sync.dma_start(zall[hh:hh + 1, :], stg[32:33, :])
                zf = rowp.tile([8, S], F32, tag="zrtmp")
                nc.vector.tensor_copy(zf[:], zall[:])
                zrtmp = rowp.tile([8, S], F32, tag="zrtmp")
                nc.vector.reciprocal_approx_fast(zrtmp[:], zf[:])
                nc.vector.tensor_copy(zrall[:], zrtmp[:])
                for c in range(NKT):
                    zrep = ps.tile([128, S], F32, tag="pv")
                    mm_halves(zrep, sel_t[:, c * 128:(c + 1) * 128],
                              zrall[:], start=True, stop=True)
                    nc.vector.tensor_tensor(oall[c][:], oall[c][:], zrep[:],
                                            OP.mult)
                # ---- wo + residual ----
                for c in range(NKT):
                    aps = ps.tile([128, S], F32, tag="pv")
                    for kt in range(NKT):
                        wot = wp.tile([128, 128], BF16, tag="wc")
                        nc.sync.dma_start(
                            wot[:], wo[l, kt * 128:(kt + 1) * 128,
                                       c * 128:(c + 1) * 128])
                        mm_halves(aps, wot[:], oall[kt][:], start=(kt == 0),
                                stop=(kt == NKT - 1))
                    if use_biases:
                        nc.vector.scalar_tensor_tensor(
                            xr[c][:], aps[:],
                            pcol(bo[l, c * 128:(c + 1) * 128, :])[:],
                            h[c][:], OP.add, OP.add)
                    else:
                        nc.vector.tensor_tensor(xr[c][:], aps[:], h[c][:],
                                                OP.add)
                if use_ln_affine:
                    layernorm(xr, h,
                              [ln1g[l, k * 128:(k + 1) * 128, :] for k in range(NKT)],
                              [ln1b[l, k * 128:(k + 1) * 128, :] for k in range(NKT)])
                else:
                    layernorm(xr, h, None, None)
                for kt in range(NKT):
                    nc.vector.tensor_copy(h16[kt][:], h[kt][:])

                # ---- FFN ----
                w1t = [w1p.tile([128, DFF], BF16, tag="w1", name=f"w1t{i}") for i in range(NKT)]
                for kt in range(NKT):
                    nc.sync.dma_start(w1t[kt][:],
                                      w1[l, kt * 128:(kt + 1) * 128, :])
                w2t = [w2p.tile([128, D], BF16, tag="w2", name=f"w2t{i}") for i in range(DFF // 128)]
                for kt in range(DFF // 128):
                    nc.sync.dma_start(w2t[kt][:],
                                      w2[l, kt * 128:(kt + 1) * 128, :])
                fps = [ps.tile([128, S], F32, tag="pv", name=f"fps{i}") for i in range(NKT)]
                for ch in range(DFF // 128):
                    gh = [scp.tile([128, 512], F32, tag="sc", name=f"gh{i}") for i in range(2)]
                    for kt in range(NKT):
                        for hf in range(2):
                            nc.tensor.matmul(
                                gh[hf][:],
                                w1t[kt][:, ch * 128:(ch + 1) * 128],
                                h16[kt][:, hf * 512:(hf + 1) * 512],
                                start=(kt == 0), stop=(kt == NKT - 1),
                                skip_group_check=True)
                    gt = io.tile([128, S], BF16, tag="gt")
                    b1c = (pcol(b1[l, ch * 128:(ch + 1) * 128, :])
                           if use_biases else None)
                    for hf in range(2):
                        dstg = gt[:, hf * 512:(hf + 1) * 512]
                        if ch % 2 == 0:
                            nc.scalar.activation(
                                dstg, gh[hf][:], AF.Relu,
                                bias=(b1c[:] if b1c is not None else 0.0))
                        else:
                            if b1c is not None:
                                nc.vector.tensor_scalar(
                                    dstg, gh[hf][:], b1c[:], 0.0,
                                    OP.add, OP.max)
                            else:
                                nc.vector.tensor_scalar_max(dstg, gh[hf][:],
                                                            0.0)
                    for c in range(NKT):
                        mm_halves(fps[c], w2t[ch][:, c * 128:(c + 1) * 128],
                                gt[:], start=(ch == 0),
                                stop=(ch == DFF // 128 - 1))
                for c in range(NKT):
                    if use_biases:
                        nc.vector.scalar_tensor_tensor(
                            xr[c][:], fps[c][:],
                            pcol(b2[l, c * 128:(c + 1) * 128, :])[:],
                            h[c][:], OP.add, OP.add)
                    else:
                        nc.vector.tensor_tensor(xr[c][:], fps[c][:], h[c][:],
                                                OP.add)
                if use_ln_affine:
                    layernorm(xr, h,
                              [ln2g[l, k * 128:(k + 1) * 128, :] for k in range(NKT)],
                              [ln2b[l, k * 128:(k + 1) * 128, :] for k in range(NKT)])
                else:
                    layernorm(xr, h, None, None)
                for kt in range(NKT):
                    nc.vector.tensor_copy(h16[kt][:], h[kt][:])

            # ================= final LN + classifier =================
            hf_t = [msc.tile([128, S], F32R, tag="sq", name=f"hf{i}") for i in range(NKT)]
            if use_ln_affine:
                layernorm(h, hf_t,
                          [lnfg[k * 128:(k + 1) * 128, :] for k in range(NKT)],
                          [lnfb[k * 128:(k + 1) * 128, :] for k in range(NKT)])
            else:
                layernorm(h, hf_t, None, None)
            cps = ps.tile([NCLS * PP * PP, S], F32, tag="pv")
            for kt in range(NKT):
                cwt = wp.tile([128, NCLS * PP * PP], F32R, tag="wc")
                nc.sync.dma_start(cwt[:], cls_w[kt * 128:(kt + 1) * 128, :])
                mm_halves(cps, cwt[:], hf_t[kt][:], start=(kt == 0),
                          stop=(kt == NKT - 1))
            outt = io.tile([NCLS * PP * PP, S], F32, tag="io")
            if use_biases:
                nc.scalar.activation(outt[:], cps[:], AF.Identity,
                                     bias=pcol(clsb[:])[:])
            else:
                nc.scalar.copy(outt[:], cps[:])
            nc.sync.dma_start(out_pl[:], outt[:])


def _prep_host(inputs):
    f = lambda a: np.ascontiguousarray(np.asarray(a), dtype=np.float32)
    x = f(inputs["x"])
    conv_w = f(inputs["conv_w"])
    pos = f(inputs["pos_embed"])
    rpb = f(inputs["rpb"])

    xs = []
    for b in range(B):
        xb = x[b].reshape(C_IN, IMG // PP, PP, IMG // PP, PP)
        xb = xb.transpose(0, 2, 4, 1, 3).reshape(KIN, S)
        xp = np.zeros((KIN_PAD, S), np.float32)
        xp[:KIN] = xb
        xs.append(xp)

    w = {}
    cw = conv_w.reshape(D, C_IN, PP, PP).transpose(1, 2, 3, 0).reshape(KIN, D)
    cwp = np.zeros((KIN_PAD, D), np.float32)
    cwp[:KIN] = cw
    w["conv_w"] = cwp
    w["pos_t"] = f(pos.reshape(S, D).T)
    scale = 1.0 / np.sqrt(np.float32(HD))
    import ml_dtypes
    bf = lambda a: np.ascontiguousarray(a).astype(ml_dtypes.bfloat16)
    w["wq"] = bf(np.transpose(f(inputs["wq"]), (0, 2, 1)) * scale)
    w["wk"] = bf(np.transpose(f(inputs["wk"]), (0, 2, 1)))
    w["wv"] = bf(np.transpose(f(inputs["wv"]), (0, 2, 1)))
    w["wo"] = bf(np.transpose(f(inputs["wo"]), (0, 2, 1)))
    w["w1"] = bf(np.transpose(f(inputs["w1"]), (0, 2, 1)))
    w["w2"] = bf(np.transpose(f(inputs["w2"]), (0, 2, 1)))
    bc = np.zeros((L, NH, 128, 1920), np.float32)
    for l in range(L):
        for hh in range(NH):
            th = np.ascontiguousarray(rpb[l, :, hh])
            bc[l, hh] = np.lib.stride_tricks.as_strided(
                th[127:], shape=(128, 1920), strides=(-4, 4))
    w["bcache"] = bc.astype(ml_dtypes.bfloat16)
    w["ident"] = np.eye(128, dtype=np.float32).astype(ml_dtypes.bfloat16)
    w["ident32"] = np.eye(128, dtype=np.float32)
    w["ones1"] = np.ones((1, 128), np.float32)
    w["oavgc"] = np.full((128, 1), 1.0 / D, np.float32)
    selw = np.zeros((8, 2 * 128), np.float32)
    for c in range(2):
        for p in range(128):
            selw[4 * c + p // 32, c * 128 + p] = 1.0
    w["sel"] = selw
    w["cls_w"] = f(inputs["cls_w"].T)
    vinit = np.zeros((128, NST * VSTRIDE), np.float32)
    for st in range(NST):
        for hh in range(NH):
            vinit[:, st * VSTRIDE + hh * (HD + 1) + HD] = 1.0
    w["vinit"] = vinit.astype(ml_dtypes.bfloat16)

    use_biases = any(
        np.abs(f(inputs[k])).max() > 0
        for k in ("bq", "bk", "bv", "bo", "b1", "b2", "conv_b", "cls_b"))
    use_ln_affine = not (
        np.allclose(f(inputs["ln1_s"]), 1.0)
        and np.allclose(f(inputs["ln2_s"]), 1.0)
        and np.allclose(f(inputs["lnf_s"]), 1.0)
        and np.abs(f(inputs["ln1_b"])).max() == 0
        and np.abs(f(inputs["ln2_b"])).max() == 0
        and np.abs(f(inputs["lnf_b"])).max() == 0)
    if use_biases:
        w["bq"] = f(inputs["bq"]).reshape(L, D, 1)
        w["bk"] = f(inputs["bk"]).reshape(L, D, 1)
        w["bv"] = np.ascontiguousarray(
            np.broadcast_to(f(inputs["bv"])[:, None, :], (L, 128, D)))
        w["bo"] = f(inputs["bo"]).reshape(L, D, 1)
        w["b1"] = f(inputs["b1"]).reshape(L, DFF, 1)
        w["b2"] = f(inputs["b2"]).reshape(L, D, 1)
        w["convb"] = f(inputs["conv_b"]).reshape(D, 1)
        w["clsb"] = f(inputs["cls_b"]).reshape(NCLS * PP * PP, 1)
    if use_ln_affine:
        w["ln1g"] = f(inputs["ln1_s"]).reshape(L, D, 1)
        w["ln1b"] = f(inputs["ln1_b"]).reshape(L, D, 1)
        w["ln2g"] = f(inputs["ln2_s"]).reshape(L, D, 1)
        w["ln2b"] = f(inputs["ln2_b"]).reshape(L, D, 1)
        w["lnfg"] = f(inputs["lnf_s"]).reshape(D, 1)
        w["lnfb"] = f(inputs["lnf_b"]).reshape(D, 1)
    return w, xs, use_ln_affine, use_biases


_RUN_KWARGS = {}


def kernel(**inputs):
    w, xs, use_ln_affine, use_biases = _prep_host(inputs)
    nc = bacc.Bacc("TRN2")
    _build(nc, use_ln_affine, use_biases)
    nc.finalize()
    in_maps = [dict(w, x_unf=xs[b]) for b in range(B)]
    res = run_bass_kernel_spmd(nc, in_maps, core_ids=list(range(B)),
                               **_RUN_KWARGS)
    kernel.last_result = res
    out = np.empty((B, NCLS, IMG, IMG), np.float32)
    for b in range(B):
        pl = res.results[b]["out_pl"]
        pl = pl.reshape(NCLS, PP, PP, IMG // PP, IMG // PP)
        out[b] = pl.transpose(0, 3, 1, 4, 2).reshape(NCLS, IMG, IMG)
    return out


# revision 10
# speedup vs baseline: 1.1996x; 1.0856x over previous
"""Trainium2 Bass kernel for nn_CRITTransformer (ViT-style dense transformer).

kernel(**inputs) takes FULL inputs as in reference.setup_inputs() and returns
the FULL [8, 6, 128, 128] output. Data-parallel over batch across 8
NeuronCores (1 image per core), weights replicated.

Per-core layout:
  - activations transposed: h[d=256 (2 k-tiles), s=1024]
  - attention scores^T[k, q] tiles; softmax denominators via ones-column
    appended to V (PV matmul row 32); no partition reductions
  - relative-position bias: host-expanded [128, 1920] Toeplitz cache per
    (layer, head); any k-tile's bias block is a contiguous column window;
    accumulated into scores PSUM via identity matmul
  - matmuls in float32r (full PE rate at N>=256, ~12-bit mantissa)
  - LayerNorm: stats via ones-column matmuls; rstd = exp(-0.5*ln(var+eps))
    (stays in the natural_log_exp ACT table set); stats replicated across
    partitions with K=1 matmuls
"""

import numpy as np

import concourse.bass as bass
import concourse.mybir as mybir
import concourse.tile as tile
from concourse import bacc
from concourse.bass_utils import run_bass_kernel_spmd

F32R = mybir.dt.float32r
F32 = mybir.dt.float32
BF16 = mybir.dt.bfloat16
AF = mybir.ActivationFunctionType
OP = mybir.AluOpType

B, C_IN, IMG, PP, D, NH, L, DFF, NCLS, MAXS = 8, 42, 128, 4, 256, 8, 4, 1024, 6, 1024
S = (IMG // PP) ** 2   # 1024
HD = D // NH           # 32
KIN = C_IN * PP * PP   # 672
KIN_PAD = 768
NKT = D // 128         # 2
NST = S // 128         # 8
VSTRIDE = NH * (HD + 1)  # 264 per s-tile in vall
EPS = 1e-6


def _build(nc, use_ln_affine, use_biases):
    def din(name, shape, dtype=F32R):
        return nc.dram_tensor(name, shape, dtype, kind="ExternalInput")

    x_unf = din("x_unf", [KIN_PAD, S])
    conv_w = din("conv_w", [KIN_PAD, D])
    pos_t = din("pos_t", [D, S])
    wq = din("wq", [L, D, D], BF16)
    wk = din("wk", [L, D, D], BF16)
    wv = din("wv", [L, D, D], BF16)
    wo = din("wo", [L, D, D], BF16)
    w1 = din("w1", [L, D, DFF], BF16)
    w2 = din("w2", [L, DFF, D], BF16)
    bcache = din("bcache", [L, NH, 128, 1920], BF16)
    ident = din("ident", [128, 128], BF16)
    ident32 = din("ident32", [128, 128])
    ones1 = din("ones1", [1, 128])
    oavgc = din("oavgc", [128, 1])
    sel = din("sel", [8, 2 * 128])
    vinit = din("vinit", [128, NST * VSTRIDE], BF16)
    cls_w = din("cls_w", [D, NCLS * PP * PP])
    if use_biases:
        bq = din("bq", [L, D, 1], F32)
        bk = din("bk", [L, D, 1], F32)
        bv = din("bv", [L, 128, D], F32)
        bo = din("bo", [L, D, 1], F32)
        b1 = din("b1", [L, DFF, 1], F32)
        b2 = din("b2", [L, D, 1], F32)
        convb = din("convb", [D, 1], F32)
        clsb = din("clsb", [NCLS * PP * PP, 1], F32)
    if use_ln_affine:
        ln1g = din("ln1g", [L, D, 1], F32)
        ln1b = din("ln1b", [L, D, 1], F32)
        ln2g = din("ln2g", [L, D, 1], F32)
        ln2b = din("ln2b", [L, D, 1], F32)
        lnfg = din("lnfg", [D, 1], F32)
        lnfb = din("lnfb", [D, 1], F32)

    out_pl = nc.dram_tensor("out_pl", [NCLS * PP * PP, S], F32,
                            kind="ExternalOutput")

    with tile.TileContext(nc) as tc:
        with (
            tc.tile_pool(name="res", bufs=1) as res,
            tc.tile_pool(name="io", bufs=3) as io,
            tc.tile_pool(name="wp", bufs=6) as wp,
            tc.tile_pool(name="w1p", bufs=3) as w1p,
            tc.tile_pool(name="w2p", bufs=9) as w2p,
            tc.tile_pool(name="bcp", bufs=3) as bcp,
            tc.tile_pool(name="ep", bufs=4) as ep,
            tc.tile_pool(name="sgp", bufs=2) as sgp,
            tc.tile_pool(name="rowp", bufs=4) as rowp,
            tc.tile_pool(name="msc", bufs=3) as msc,
            tc.tile_pool(name="pcl", bufs=4) as pcl,
            tc.tile_pool(name="ps", bufs=2, space="PSUM") as ps,      # 2x4KB
            tc.tile_pool(name="ps2", bufs=2, space="PSUM") as ps2,    # 2x4KB
        ):
            # ---- constants ----
            ident_t = res.tile([128, 128], BF16, tag="ident")
            nc.sync.dma_start(ident_t[:], ident[:])
            ident32_t = res.tile([128, 128], F32R, tag="ident32")
            nc.sync.dma_start(ident32_t[:], ident32[:])
            ones1_t = res.tile([1, 128], F32R, tag="ones1")
            nc.sync.dma_start(ones1_t[:], ones1[:])
            oavgc_t = res.tile([128, 1], F32R, tag="oavgc")
            nc.sync.dma_start(oavgc_t[:], oavgc[:])
            sel_t = res.tile([8, 2 * 128], F32R, tag="sel")
            nc.sync.dma_start(sel_t[:], sel[:])
            epst = res.tile([128, 1], F32, tag="eps")
            nc.vector.memset(epst[:], EPS)

            h = [res.tile([128, S], F32R, tag=f"h{kt}", name=f"h{kt}") for kt in range(NKT)]
            h16 = [res.tile([128, S], BF16, tag=f"h16{kt}", name=f"h16_{kt}") for kt in range(NKT)]
            xr = [res.tile([128, S], F32R, tag=f"xr{kt}", name=f"xr{kt}") for kt in range(NKT)]
            qt = [res.tile([128, S], BF16, tag=f"qt{c}", name=f"qt{c}") for c in range(NKT)]
            ktsb = [res.tile([128, S], BF16, tag=f"kt{c}", name=f"ktsb{c}") for c in range(NKT)]
            oall = [res.tile([128, S], BF16, tag=f"oall{c}", name=f"oall{c}") for c in range(NKT)]
            vall = res.tile([128, NST * VSTRIDE], BF16, tag="vall")
            zall = res.tile([8, S], BF16, tag="zall")
            zrall = res.tile([8, S], F32R, tag="zrall")

            nc.sync.dma_start(vall[:], vinit[:])

            def mm_halves(psum, lhsT, rhs, start, stop, tile_position=None):
                for hf in range(2):
                    nc.tensor.matmul(
                        psum[:, hf * 512:(hf + 1) * 512], lhsT,
                        rhs[:, hf * 512:(hf + 1) * 512],
                        start=start, stop=stop, skip_group_check=True,
                        tile_position=tile_position)


            def pcol(src_ap):
                t = pcl.tile([128, 1], F32, tag="pcol", name="pcol")
                nc.sync.dma_start(t[:src_ap.shape[0], :], src_ap)
                return t

            # ================= patch embedding =================
            for c in range(NKT):
                cps = ps.tile([128, S], F32, tag="pv")
                for kt in range(6):
                    xt_ = io.tile([128, S], F32R, tag="io")
                    nc.sync.dma_start(xt_[:], x_unf[kt * 128:(kt + 1) * 128, :])
                    wt = wp.tile([128, 128], F32R, tag="wc")
                    nc.sync.dma_start(
                        wt[:], conv_w[kt * 128:(kt + 1) * 128,
                                      c * 128:(c + 1) * 128])
                    mm_halves(cps, wt[:], xt_[:], start=(kt == 0), stop=False)
                post = io.tile([128, S], F32R, tag="io")
                nc.sync.dma_start(post[:], pos_t[c * 128:(c + 1) * 128, :])
                mm_halves(cps, ident32_t[:], post[:], start=False, stop=True)
                if use_biases:
                    nc.scalar.activation(h[c][:], cps[:], AF.Identity,
                                         bias=pcol(convb[c * 128:(c + 1) * 128, :])[:])
                else:
                    nc.scalar.copy(h[c][:], cps[:])
                nc.vector.tensor_copy(h16[c][:], h[c][:])

            # ================= layernorm helper =================
            def layernorm(xt, out_t, g_ap, b_ap):
                mrow_ps = ps.tile([1, S], F32, tag="pv")
                qrow_ps = ps.tile([1, S], F32, tag="pv")
                for kt in range(NKT):
                    sq = msc.tile([128, S], F32R, tag="sq")
                    nc.vector.tensor_tensor(sq[:], xt[kt][:], xt[kt][:],
                                            OP.mult)
                    mm_halves(mrow_ps, oavgc_t[:], xt[kt][:],
                              start=(kt == 0), stop=(kt == NKT - 1))
                    mm_halves(qrow_ps, oavgc_t[:], sq[:],
                              start=(kt == 0), stop=(kt == NKT - 1))
                mrow = rowp.tile([1, S], F32R, tag="row")
                nc.vector.tensor_copy(mrow[:], mrow_ps[:])
                m2 = rowp.tile([1, S], F32, tag="row")
                nc.vector.tensor_tensor(m2[:], mrow[:], mrow[:], OP.mult)
                var = rowp.tile([1, S], F32, tag="row")
                nc.vector.tensor_tensor(var[:], qrow_ps[:], m2[:], OP.subtract)
                rrow = rowp.tile([1, S], F32R, tag="row")
                nc.scalar.activation(rrow[:], var[:], AF.Ln, bias=epst[0:1, :])
                nc.scalar.activation(rrow[:], rrow[:], AF.Exp, scale=-0.5)
                arow = rowp.tile([1, S], F32R, tag="row")
                nc.vector.scalar_tensor_tensor(arow[:], mrow[:], -1.0,
                                               rrow[:], OP.mult, OP.mult)
                rrep = ps.tile([128, S], F32, tag="pv")
                mm_halves(rrep, ones1_t[:], rrow[:], start=True, stop=True)
                arep = ps.tile([128, S], F32, tag="pv")
                mm_halves(arep, ones1_t[:], arow[:], start=True, stop=True)
                for kt in range(NKT):
                    u = msc.tile([128, S], F32, tag="sq")
                    nc.vector.tensor_tensor(u[:], xt[kt][:], rrep[:], OP.mult)
                    if g_ap is None:
                        nc.vector.tensor_tensor(out_t[kt][:], u[:], arep[:],
                                                OP.add)
                    else:
                        u2 = msc.tile([128, S], F32, tag="sq")
                        nc.vector.tensor_tensor(u2[:], u[:], arep[:], OP.add)
                        nc.scalar.activation(out_t[kt][:], u2[:], AF.Identity,
                                             scale=pcol(g_ap[kt])[:],
                                             bias=pcol(b_ap[kt])[:])

            # ================= transformer layers =================
            for l in range(L):
                # ---- Q^T, K^T ----
                for c in range(NKT):
                    qps = ps.tile([128, S], F32, tag="pv")
                    kps = ps.tile([128, S], F32, tag="pv")
                    for kt in range(NKT):
                        wqt = wp.tile([128, 128], BF16, tag="wc")
                        nc.sync.dma_start(
                            wqt[:], wq[l, kt * 128:(kt + 1) * 128,
                                       c * 128:(c + 1) * 128])
                        mm_halves(qps, wqt[:], h16[kt][:], start=(kt == 0),
                                stop=(kt == NKT - 1))
                        wkt = wp.tile([128, 128], BF16, tag="wc")
                        nc.sync.dma_start(
                            wkt[:], wk[l, kt * 128:(kt + 1) * 128,
                                       c * 128:(c + 1) * 128])
                        mm_halves(kps, wkt[:], h16[kt][:], start=(kt == 0),
                                stop=(kt == NKT - 1))
                    if use_biases:
                        nc.scalar.activation(
                            qt[c][:], qps[:], AF.Identity,
                            bias=pcol(bq[l, c * 128:(c + 1) * 128, :])[:])
                        nc.scalar.activation(
                            ktsb[c][:], kps[:], AF.Identity,
                            bias=pcol(bk[l, c * 128:(c + 1) * 128, :])[:])
                    else:
                        nc.scalar.copy(qt[c][:], qps[:])
                        nc.vector.tensor_copy(ktsb[c][:], kps[:])
                # ---- V (s-partition layout, interleaved ones cols) ----
                wvt = [w1p.tile([128, D], BF16, tag="wv", name=f"wv{i}") for i in range(NKT)]
                for kt in range(NKT):
                    nc.sync.dma_start(wvt[kt][:],
                                      wv[l, kt * 128:(kt + 1) * 128, :])
                if use_biases:
                    bvt = msc.tile([128, D], F32, tag="bvrep")
                    nc.sync.dma_start(bvt[:], bv[l])
                for st in range(NST):
                    vps = ps2.tile([128, D], F32, tag="mm2", name="vps")
                    for kt in range(NKT):
                        nc.tensor.matmul(
                            vps[:], h16[kt][:, st * 128:(st + 1) * 128],
                            wvt[kt][:], start=(kt == 0),
                            stop=(kt == NKT - 1), skip_group_check=True)
                    base = st * VSTRIDE
                    dst = bass.AP(vall.tensor, vall[:].offset + base,
                                  [list(vall[:].ap[0]), [HD + 1, NH], [1, HD]])
                    if use_biases:
                        nc.vector.tensor_tensor(
                            dst, vps[:].rearrange("p (a b) -> p a b", a=NH),
                            bvt[:].rearrange("p (a b) -> p a b", a=NH), OP.add)
                    else:
                        nc.vector.tensor_copy(
                            dst, vps[:].rearrange("p (a b) -> p a b", a=NH))

                # ---- attention ----
                for g in range(4):          # head pairs
                    h0 = 2 * g
                    chunk = h0 // 4
                    r0 = 32 * (h0 % 4)
                    bct = []
                    for j in range(2):
                        t = bcp.tile([128, 1920], BF16, tag="bc", name="bct")
                        nc.sync.dma_start(t[:], bcache[l, 2 * g + j])
                        bct.append(t)
                    pvps = [ps.tile([33, S], F32, tag="pv", name=f"pv{i}")
                            for i in range(2)]
                    for kt8 in range(NST):
                        scts = []
                        for j in range(2):
                            jr = r0 + 32 * j
                            sct = ps2.tile([128, S], F32, tag="mm2",
                                           name="sct")
                            for qh in range(2):
                                nc.tensor.matmul(
                                    sct[:, qh * 512:(qh + 1) * 512],
                                    ktsb[chunk][jr:jr + 32,
                                                kt8 * 128:(kt8 + 1) * 128],
                                    qt[chunk][jr:jr + 32,
                                              qh * 512:(qh + 1) * 512],
                                    start=True, stop=True,
                                    skip_group_check=True,
                                    tile_position=(jr, 0))
                            scts.append(sct)
                        for j in range(2):
                            hh = 2 * g + j
                            et = ep.tile([128, S], BF16, tag="e", name="et")
                            nc.scalar.activation(et[:], scts[j][:], AF.Exp)
                            nc.vector.tensor_tensor(
                                et[:], et[:],
                                bct[j][:, (7 - kt8) * 128:
                                       (7 - kt8) * 128 + S],
                                OP.mult)
                            vsl = vall[:, kt8 * VSTRIDE + hh * (HD + 1):
                                       kt8 * VSTRIDE + hh * (HD + 1) + HD + 1]
                            for qh in range(2):
                                nc.tensor.matmul(
                                    pvps[j][:, qh * 512:(qh + 1) * 512],
                                    vsl, et[:, qh * 512:(qh + 1) * 512],
                                    start=(kt8 == 0), stop=(kt8 == NST - 1),
                                    skip_group_check=True)
                    for j in range(2):
                        hh = 2 * g + j
                        stg = sgp.tile([33, S], BF16, tag="stage", name="stg")
                        nc.vector.tensor_copy(stg[:], pvps[j][:])
                        nc.sync.dma_start(
                            oall[hh // 4][32 * (hh % 4):32 * (hh % 4) + 32, :],
                            stg[0:32, :])
                        nc.sync.dma_start(zall[hh:hh + 1, :], stg[32:33, :])
                zf = rowp.tile([8, S], F32, tag="zrtmp")
                nc.vector.tensor_copy(zf[:], zall[:])
                zrtmp = rowp.tile([8, S], F32, tag="zrtmp")
                nc.vector.reciprocal_approx_fast(zrtmp[:], zf[:])
                nc.vector.tensor_copy(zrall[:], zrtmp[:])
                for c in range(NKT):
                    zrep = ps.tile([128, S], F32, tag="pv")
                    mm_halves(zrep, sel_t[:, c * 128:(c + 1) * 128],
                              zrall[:], start=True, stop=True)
                    nc.vector.tensor_tensor(oall[c][:], oall[c][:], zrep[:],
                                            OP.mult)
                # ---- wo + residual ----
                for c in range(NKT):
                    aps = ps.tile([128, S], F32, tag="pv")
                    for kt in range(NKT):
                        wot = wp.tile([128, 128], BF16, tag="wc")
                        nc.sync.dma_start(
                            wot[:], wo[l, kt * 128:(kt + 1) * 128,
                                       c * 128:(c + 1) * 128])
                        mm_halves(aps, wot[:], oall[kt][:], start=(kt == 0),
                                stop=(kt == NKT - 1))
                    if use_biases:
                        nc.vector.scalar_tensor_tensor(
                            xr[c][:], aps[:],
                            pcol(bo[l, c * 128:(c + 1) * 128, :])[:],
                            h[c][:], OP.add, OP.add)
                    else:
                        nc.vector.tensor_tensor(xr[c][:], aps[:], h[c][:],
                                                OP.add)
                if use_ln_affine:
                    layernorm(xr, h,
                              [ln1g[l, k * 128:(k + 1) * 128, :] for k in range(NKT)],
                              [ln1b[l, k * 128:(k + 1) * 128, :] for k in range(NKT)])
                else:
                    layernorm(xr, h, None, None)
                for kt in range(NKT):
                    nc.vector.tensor_copy(h16[kt][:], h[kt][:])

                # ---- FFN ----
                w1t = [w1p.tile([128, DFF], BF16, tag="w1", name=f"w1t{i}") for i in range(NKT)]
                for kt in range(NKT):
                    nc.sync.dma_start(w1t[kt][:],
                                      w1[l, kt * 128:(kt + 1) * 128, :])
                w2t = [w2p.tile([128, D], BF16, tag="w2", name=f"w2t{i}") for i in range(DFF // 128)]
                for kt in range(DFF // 128):
                    nc.sync.dma_start(w2t[kt][:],
                                      w2[l, kt * 128:(kt + 1) * 128, :])
                fps = [ps.tile([128, S], F32, tag="pv", name=f"fps{i}") for i in range(NKT)]
                for ch in range(DFF // 128):
                    gps = ps2.tile([128, S], F32, tag="mm2", name="gps")
                    for kt in range(NKT):
                        mm_halves(gps, w1t[kt][:, ch * 128:(ch + 1) * 128],
                                  h16[kt][:], start=(kt == 0),
                                  stop=(kt == NKT - 1))
                    gt = io.tile([128, S], BF16, tag="gt")
                    b1c = (pcol(b1[l, ch * 128:(ch + 1) * 128, :])
                           if use_biases else None)
                    if ch % 2 == 0:
                        nc.scalar.activation(
                            gt[:], gps[:], AF.Relu,
                            bias=(b1c[:] if b1c is not None else 0.0))
                    else:
                        if b1c is not None:
                            nc.vector.tensor_scalar(
                                gt[:], gps[:], b1c[:], 0.0, OP.add, OP.max)
                        else:
                            nc.vector.tensor_scalar_max(gt[:], gps[:], 0.0)
                    for c in range(NKT):
                        mm_halves(fps[c], w2t[ch][:, c * 128:(c + 1) * 128],
                                  gt[:], start=(ch == 0),
                                  stop=(ch == DFF // 128 - 1))
                for c in range(NKT):
                    if use_biases:
                        nc.vector.scalar_tensor_tensor(
                            xr[c][:], fps[c][:],
                            pcol(b2[l, c * 128:(c + 1) * 128, :])[:],
                            h[c][:], OP.add, OP.add)
                    else:
                        nc.vector.tensor_tensor(xr[c][:], fps[c][:], h[c][:],
                                                OP.add)
                if use_ln_affine:
                    layernorm(xr, h,
                              [ln2g[l, k * 128:(k + 1) * 128, :] for k in range(NKT)],
                              [ln2b[l, k * 128:(k + 1) * 128, :] for k in range(NKT)])
                else:
                    layernorm(xr, h, None, None)
                for kt in range(NKT):
                    nc.vector.tensor_copy(h16[kt][:], h[kt][:])

            # ================= final LN + classifier =================
            hf_t = [msc.tile([128, S], F32R, tag="sq", name=f"hf{i}") for i in range(NKT)]
            if use_ln_affine:
                layernorm(h, hf_t,
                          [lnfg[k * 128:(k + 1) * 128, :] for k in range(NKT)],
                          [lnfb[k * 128:(k + 1) * 128, :] for k in range(NKT)])
            else:
                layernorm(h, hf_t, None, None)
            cps = ps.tile([NCLS * PP * PP, S], F32, tag="pv")
            for kt in range(NKT):
                cwt = wp.tile([128, NCLS * PP * PP], F32R, tag="wc")
                nc.sync.dma_start(cwt[:], cls_w[kt * 128:(kt + 1) * 128, :])
                mm_halves(cps, cwt[:], hf_t[kt][:], start=(kt == 0),
                          stop=(kt == NKT - 1))
            outt = io.tile([NCLS * PP * PP, S], F32, tag="io")
            if use_biases:
                nc.scalar.activation(outt[:], cps[:], AF.Identity,
                                     bias=pcol(clsb[:])[:])
            else:
                nc.scalar.copy(outt[:], cps[:])
            nc.sync.dma_start(out_pl[:], outt[:])


def _prep_host(inputs):
    f = lambda a: np.ascontiguousarray(np.asarray(a), dtype=np.float32)
    x = f(inputs["x"])
    conv_w = f(inputs["conv_w"])
    pos = f(inputs["pos_embed"])
    rpb = f(inputs["rpb"])

    xs = []
    for b in range(B):
        xb = x[b].reshape(C_IN, IMG // PP, PP, IMG // PP, PP)
        xb = xb.transpose(0, 2, 4, 1, 3).reshape(KIN, S)
        xp = np.zeros((KIN_PAD, S), np.float32)
        xp[:KIN] = xb
        xs.append(xp)

    w = {}
    cw = conv_w.reshape(D, C_IN, PP, PP).transpose(1, 2, 3, 0).reshape(KIN, D)
    cwp = np.zeros((KIN_PAD, D), np.float32)
    cwp[:KIN] = cw
    w["conv_w"] = cwp
    w["pos_t"] = f(pos.reshape(S, D).T)
    scale = 1.0 / np.sqrt(np.float32(HD))
    import ml_dtypes
    bf = lambda a: np.ascontiguousarray(a).astype(ml_dtypes.bfloat16)
    w["wq"] = bf(np.transpose(f(inputs["wq"]), (0, 2, 1)) * scale)
    w["wk"] = bf(np.transpose(f(inputs["wk"]), (0, 2, 1)))
    w["wv"] = bf(np.transpose(f(inputs["wv"]), (0, 2, 1)))
    w["wo"] = bf(np.transpose(f(inputs["wo"]), (0, 2, 1)))
    w["w1"] = bf(np.transpose(f(inputs["w1"]), (0, 2, 1)))
    w["w2"] = bf(np.transpose(f(inputs["w2"]), (0, 2, 1)))
    bc = np.zeros((L, NH, 128, 1920), np.float32)
    for l in range(L):
        for hh in range(NH):
            th = np.ascontiguousarray(rpb[l, :, hh])
            bc[l, hh] = np.lib.stride_tricks.as_strided(
                th[127:], shape=(128, 1920), strides=(-4, 4))
    w["bcache"] = np.exp(bc).astype(ml_dtypes.bfloat16)
    w["ident"] = np.eye(128, dtype=np.float32).astype(ml_dtypes.bfloat16)
    w["ident32"] = np.eye(128, dtype=np.float32)
    w["ones1"] = np.ones((1, 128), np.float32)
    w["oavgc"] = np.full((128, 1), 1.0 / D, np.float32)
    selw = np.zeros((8, 2 * 128), np.float32)
    for c in range(2):
        for p in range(128):
            selw[4 * c + p // 32, c * 128 + p] = 1.0
    w["sel"] = selw
    w["cls_w"] = f(inputs["cls_w"].T)
    vinit = np.zeros((128, NST * VSTRIDE), np.float32)
    for st in range(NST):
        for hh in range(NH):
            vinit[:, st * VSTRIDE + hh * (HD + 1) + HD] = 1.0
    w["vinit"] = vinit.astype(ml_dtypes.bfloat16)

    use_biases = any(
        np.abs(f(inputs[k])).max() > 0
        for k in ("bq", "bk", "bv", "bo", "b1", "b2", "conv_b", "cls_b"))
    use_ln_affine = not (
        np.allclose(f(inputs["ln1_s"]), 1.0)
        and np.allclose(f(inputs["ln2_s"]), 1.0)
        and np.allclose(f(inputs["lnf_s"]), 1.0)
        and np.abs(f(inputs["ln1_b"])).max() == 0
        and np.abs(f(inputs["ln2_b"])).max() == 0
        and np.abs(f(inputs["lnf_b"])).max() == 0)
    if use_biases:
        w["bq"] = f(inputs["bq"]).reshape(L, D, 1)
        w["bk"] = f(inputs["bk"]).reshape(L, D, 1)
        w["bv"] = np.ascontiguousarray(
            np.broadcast_to(f(inputs["bv"])[:, None, :], (L, 128, D)))
        w["bo"] = f(inputs["bo"]).reshape(L, D, 1)
        w["b1"] = f(inputs["b1"]).reshape(L, DFF, 1)
        w["b2"] = f(inputs["b2"]).reshape(L, D, 1)
        w["convb"] = f(inputs["conv_b"]).reshape(D, 1)
        w["clsb"] = f(inputs["cls_b"]).reshape(NCLS * PP * PP, 1)
    if use_ln_affine:
        w["ln1g"] = f(inputs["ln1_s"]).reshape(L, D, 1)
        w["ln1b"] = f(inputs["ln1_b"]).reshape(L, D, 1)
        w["ln2g"] = f(inputs["ln2_s"]).reshape(L, D, 1)
        w["ln2b"] = f(inputs["ln2_b"]).reshape(L, D, 1)
        w["lnfg"] = f(inputs["lnf_s"]).reshape(D, 1)
        w["lnfb"] = f(inputs["lnf_b"]).reshape(D, 1)
    return w, xs, use_ln_affine, use_biases


_RUN_KWARGS = {}


def kernel(**inputs):
    w, xs, use_ln_affine, use_biases = _prep_host(inputs)
    nc = bacc.Bacc("TRN2")
    _build(nc, use_ln_affine, use_biases)
    nc.finalize()
    in_maps = [dict(w, x_unf=xs[b]) for b in range(B)]
    res = run_bass_kernel_spmd(nc, in_maps, core_ids=list(range(B)),
                               **_RUN_KWARGS)
    kernel.last_result = res
    out = np.empty((B, NCLS, IMG, IMG), np.float32)
    for b in range(B):
        pl = res.results[b]["out_pl"]
        pl = pl.reshape(NCLS, PP, PP, IMG // PP, IMG // PP)
        out[b] = pl.transpose(0, 3, 1, 4, 2).reshape(NCLS, IMG, IMG)
    return out
